# revision 21
# baseline (speedup 1.0000x reference)
"""Trainium2 Bass kernel for nn_EncoderLayer (attention + top-1 Switch MoE).

Strategy (8 NeuronCores):
  Launch 1 (data-parallel over batch): per core one batch row — LN1, QKV in
    transposed (feature-major) layout, flash-style attention with padded keys
    skipped, out-proj + residual, LN2. fp32r matmuls (full PE rate, ~1e-4).
  Host: router softmax/argmax (the "all-to-all" token dispatch).
  Launch 2 (expert-parallel): core e runs expert e's FFN on its routed
    tokens (capacity 1664 = 13*128), gate-scaled. Host scatter-adds back.
"""

import os
import numpy as np

os.environ.setdefault("MYCRO_LOCAL_CACHE", "1")

B, S, D, H, E, F = 8, 2048, 1024, 16, 8, 4096
DH = D // H          # 64
SR = (3 * S) // 4    # 1536 real tokens per sequence
NCORE = 8
CAP = 1664           # expert capacity (13*128)
EPS = 1e-5

_CACHE = {}


def _builders():
    import concourse.bass as bass
    import concourse.tile as tile
    from concourse import bacc, mybir
    from concourse.masks import make_identity

    f32 = mybir.dt.float32
    f32r = mybir.dt.float32r
    f16 = mybir.dt.float16
    AF = mybir.ActivationFunctionType
    ALU = mybir.AluOpType
    return bass, tile, bacc, mybir, make_identity, f32, f32r, f16, AF, ALU


def _build_launch1():
    bass, tile, bacc, mybir, make_identity, f32, f32r, f16, AF, ALU = _builders()
    from contextlib import ExitStack

    nc = bacc.Bacc(None, target_bir_lowering=False, debug=True)

    src = nc.dram_tensor("src", [S, D], f32, kind="ExternalInput")
    srcbo = nc.dram_tensor("srcbo", [S, D], f32, kind="ExternalInput")  # src + bo
    Wq = nc.dram_tensor("Wq", [D, D], f32r, kind="ExternalInput")
    Wk = nc.dram_tensor("Wk", [D, D], f32r, kind="ExternalInput")
    Wv = nc.dram_tensor("Wv", [D, D], f32r, kind="ExternalInput")
    Wo = nc.dram_tensor("Wo", [D, D], f32r, kind="ExternalInput")
    bqk = nc.dram_tensor("bqk", [128, 8, 2], f32, kind="ExternalInput")
    bv_row = nc.dram_tensor("bv_row", [1, D], f32r, kind="ExternalInput")
    src2_out = nc.dram_tensor("src2", [S, D], f32, kind="ExternalOutput")
    x2_out = nc.dram_tensor("x2", [S, D], f32, kind="ExternalOutput")
    oT_spill = nc.dram_tensor("oT_spill", [S // 128, 128, 8, 128], f32r)
    rec_dram = nc.dram_tensor("rec_dram", [8, 4, 2, 512], f32r)

    NT = S // 128            # 16 token tiles
    NKT = SR // 128          # 12 key chunks
    NQC = S // 512           # 4 q chunks

    wq_r = Wq.rearrange("(dc p) n -> p dc n", p=128)
    wk_r = Wk.rearrange("(dc p) n -> p dc n", p=128)
    wv_r = Wv.rearrange("(dc p) n -> p dc n", p=128)
    wo_r = Wo.rearrange("(dc p) n -> p dc n", p=128)

    with tile.TileContext(nc) as tc:
        with ExitStack() as ctx:
            ctx.enter_context(nc.allow_low_precision(reason="fp32r matmul pipeline"))
            persist = ctx.enter_context(tc.tile_pool(name="persist", bufs=1))
            scratch = ctx.enter_context(tc.tile_pool(name="scratch", bufs=2))
            small = ctx.enter_context(tc.tile_pool(name="small", bufs=1))
            stats_pool = ctx.enter_context(tc.tile_pool(name="stats", bufs=4))
            qpool = ctx.enter_context(tc.tile_pool(name="qpool", bufs=2))
            kpool = ctx.enter_context(tc.tile_pool(name="kpool", bufs=1))
            wvpool = ctx.enter_context(tc.tile_pool(name="wvpool", bufs=1))
            wpool = ctx.enter_context(tc.tile_pool(name="wqk", bufs=2))
            ppool = ctx.enter_context(tc.tile_pool(name="ppool", bufs=2))
            npool = ctx.enter_context(tc.tile_pool(name="npool", bufs=2))
            vpool = ctx.enter_context(tc.tile_pool(name="vpool", bufs=1))
            opool = ctx.enter_context(tc.tile_pool(name="opool", bufs=2))
            olpool = ctx.enter_context(tc.tile_pool(name="olpool", bufs=2))
            ps_pp = ctx.enter_context(tc.tile_pool(name="ps_pp", bufs=2, space="PSUM"))
            ps_s = ctx.enter_context(tc.tile_pool(name="ps_s", bufs=2, space="PSUM"))
            ps_ob = ctx.enter_context(tc.tile_pool(name="ps_ob", bufs=2, space="PSUM"))

            # ---- constants ----
            ident32 = small.tile([128, 128], f32)
            make_identity(nc, ident32)
            ident = small.tile([128, 128], f32r)
            nc.vector.tensor_copy(ident[:], ident32[:])
            ones32 = small.tile([1, 128], f32)
            nc.vector.memset(ones32[:], 1.0)
            ones_tok = small.tile([1, 128], f32r)
            nc.vector.tensor_copy(ones_tok[:], ones32[:])
            ones96 = small.tile([128, 96], f32)
            nc.vector.memset(ones96[:], 1.0)
            eps_t = small.tile([128, 1], f32)
            nc.vector.memset(eps_t[:], EPS)
            bqk_sb = small.tile([128, 8, 2], f32)
            nc.sync.dma_start(out=bqk_sb[:], in_=bqk[:])
            bv_sb = small.tile([1, D], f32r)
            nc.sync.dma_start(out=bv_sb[:], in_=bv_row[:])

            # ---- phase A: LN1 + transpose -> xT [128, 8, S] ----
            xT = persist.tile([128, 8, S], f32r)
            for t in range(NT):
                s_tile = scratch.tile([128, D], f32, tag="ld")
                nc.sync.dma_start(out=s_tile[:], in_=src[t * 128:(t + 1) * 128, :])
                st = stats_pool.tile([128, 2, 6], f32, tag="bn")
                for g in range(2):
                    nc.vector.bn_stats(out=st[:, g, :], in_=s_tile[:, g * 512:(g + 1) * 512])
                mv = stats_pool.tile([128, 2], f32, tag="mv")
                nc.vector.bn_aggr(out=mv[:], in_=st[:])
                rstd = stats_pool.tile([128, 1], f32, tag="rstd")
                nc.scalar.activation(out=rstd[:], in_=mv[:, 1:2], func=AF.Sqrt,
                                     bias=eps_t[:], scale=1.0)
                nc.vector.reciprocal(out=rstd[:], in_=rstd[:])
                x_tile = scratch.tile([128, D], f32r, tag="x")
                nc.vector.tensor_scalar(out=x_tile[:], in0=s_tile[:],
                                        scalar1=mv[:, 0:1], scalar2=rstd[:],
                                        op0=ALU.subtract, op1=ALU.mult)
                for dg in range(2):
                    pt = ps_pp.tile([128, 512], f32r, tag="pp", name="pt")
                    for d4 in range(4):
                        d = dg * 4 + d4
                        nc.tensor.transpose(pt[:, d4 * 128:(d4 + 1) * 128],
                                            x_tile[:, d * 128:(d + 1) * 128], ident[:])
                    ev_eng = nc.vector.tensor_copy if dg == 0 else nc.scalar.copy
                    ev_eng(
                        out=xT[:, dg * 4:(dg + 1) * 4, t * 128:(t + 1) * 128],
                        in_=pt.rearrange("p (a b) -> p a b", a=4))

            # ---- phase B: attention per quad / head pair ----
            for quad in range(2):
                # V + bias for 8 heads of this quad -> Vaug [128, 12, 8, 65]
                wv_sb = wvpool.tile([128, 8, 512], f32r, tag="wv")
                nc.sync.dma_start(out=wv_sb[:], in_=wv_r[:, :, quad * 512:(quad + 1) * 512])
                vaug = vpool.tile([128, NKT, 8, 65], f32r, tag="vaug")
                for vt in range(NKT):
                    ps = ps_pp.tile([128, 512], f32, tag="pp", name="ps_v")
                    for dc in range(8):
                        nc.tensor.matmul(ps[:], xT[:, dc, vt * 128:(vt + 1) * 128],
                                         wv_sb[:, dc, :], start=(dc == 0), stop=False)
                    nc.tensor.matmul(ps[:], ones_tok[:, 0:128],
                                     bv_sb[:, quad * 512:(quad + 1) * 512],
                                     start=False, stop=True)
                    nc.vector.tensor_copy(out=vaug[:, vt, :, 0:64],
                                          in_=ps.rearrange("p (h d) -> p h d", h=8))
                nc.vector.tensor_copy(
                    out=vaug[:, :, :, 64:65].rearrange("p a b one -> p (a b one)"),
                    in_=ones96[:])

                for cc in range(4):
                    c = quad * 4 + cc
                    wq_t = wpool.tile([128, 8, 128], f32r, tag="wq")
                    nc.sync.dma_start(out=wq_t[:], in_=wq_r[:, :, c * 128:(c + 1) * 128])
                    wk_t = wpool.tile([128, 8, 128], f32r, tag="wk")
                    nc.sync.dma_start(out=wk_t[:], in_=wk_r[:, :, c * 128:(c + 1) * 128])

                    qt = qpool.tile([128, S], f32r, tag="qt")
                    for qc in range(NQC):
                        ps = ps_pp.tile([128, 512], f32, tag="pp", name="ps_q")
                        for dc in range(8):
                            nc.tensor.matmul(ps[:], wq_t[:, dc, :],
                                             xT[:, dc, qc * 512:(qc + 1) * 512],
                                             start=(dc == 0), stop=(dc == 7))
                        nc.vector.tensor_scalar_add(out=qt[:, qc * 512:(qc + 1) * 512],
                                                    in0=ps[:], scalar1=bqk_sb[:, c, 0:1])
                    kt = kpool.tile([128, SR], f32r, tag="kt")
                    for kc3 in range(SR // 512):
                        ps = ps_pp.tile([128, 512], f32, tag="pp", name="ps_k")
                        for dc in range(8):
                            nc.tensor.matmul(ps[:], wk_t[:, dc, :],
                                             xT[:, dc, kc3 * 512:(kc3 + 1) * 512],
                                             start=(dc == 0), stop=(dc == 7))
                        nc.vector.tensor_scalar_add(out=kt[:, kc3 * 512:(kc3 + 1) * 512],
                                                    in0=ps[:], scalar1=bqk_sb[:, c, 1:2])

                    for qc in range(NQC):
                        po = [ps_ob.tile([128, 512], f32, tag="po", name=f"po{_h}")
                              for _h in range(2)]
                        for kc in range(NKT):
                            sp = ps_s.tile([128, 2, 512], f32, tag="ss", name="sp")
                            for half in range(2):
                                nc.tensor.matmul(
                                    sp[:, half, :],
                                    kt[64 * half:64 * half + 64, kc * 128:(kc + 1) * 128],
                                    qt[64 * half:64 * half + 64, qc * 512:(qc + 1) * 512],
                                    start=True, stop=True)
                            pt = ppool.tile([128, 2, 512], f32r, tag="pT")
                            nc.scalar.activation(
                                out=pt.rearrange("p a b -> p (a b)"),
                                in_=sp.rearrange("p a b -> p (a b)"),
                                func=AF.Exp, scale=0.125)
                            for half in range(2):
                                nc.tensor.matmul(
                                    po[half][0:65, :],
                                    vaug[:, kc, 2 * cc + half, 0:65],
                                    pt[:, half, :],
                                    start=(kc == 0), stop=(kc == NKT - 1))
                        for half in range(2):
                            raw = opool.tile([65, 512], f32r, tag="raw")
                            nc.vector.tensor_copy(out=raw[:], in_=po[half][0:65, :])
                            rs8 = npool.tile([8, 64], f32r, tag="rs8")
                            nc.sync.dma_start(out=rs8[:], in_=raw[64:65, :])
                            rr8 = npool.tile([8, 64], f32r, tag="rr8")
                            nc.vector.reciprocal(out=rr8[:], in_=rs8[:])
                            nc.sync.dma_start(out=rec_dram[c, qc, half], in_=rr8[:])
                            sbc = npool.tile([64, 512], f32r, tag="sbc")
                            nc.sync.dma_start(
                                out=sbc[:],
                                in_=rec_dram[c, qc, half].partition_broadcast(64))
                            nc.vector.tensor_mul(raw[0:64, :], raw[0:64, :], sbc[:])
                            nc.sync.dma_start(
                                out=oT_spill[4 * qc:4 * qc + 4,
                                             64 * half:64 * half + 64,
                                             c, :].rearrange("j p t -> p j t"),
                                in_=raw[0:64, :])

            # ---- phase C: out-projection + residual + LN2 ----
            wo_sb4 = vpool.tile([128, 8, D], f32r, tag="vaug", name="wo_sb4")
            nc.sync.dma_start(out=wo_sb4[:], in_=wo_r[:])
            cbuf = persist.tile([128, 8, S], f32, tag="xT", name="cbuf")
            cb = cbuf.rearrange("p a b -> p (a b)").rearrange("p (j d) -> p j d", d=D)
            for t in range(NT):
                res_tile = cb[:, 3 * (t % 4) + 0, :]
                nc.sync.dma_start(out=res_tile, in_=srcbo[t * 128:(t + 1) * 128, :])
                pss = ps_s.tile([128, 2, 512], f32, tag="ss", name="ps_op")
                ot_l = olpool.tile([128, 8, 128], f32r, tag="otl")
                nc.sync.dma_start(out=ot_l[:], in_=oT_spill[t])
                for dc in range(8):
                    for ncol in range(2):
                        nc.tensor.matmul(pss[:, ncol, :], ot_l[:, dc, :],
                                         wo_sb4[:, dc, ncol * 512:(ncol + 1) * 512],
                                         start=(dc == 0), stop=(dc == 7))
                s2_tile = cb[:, 3 * (t % 4) + 1, :]
                nc.vector.tensor_add(s2_tile,
                                     pss.rearrange("p a b -> p (a b)"),
                                     res_tile)
                nc.sync.dma_start(out=src2_out[t * 128:(t + 1) * 128, :], in_=s2_tile)
                # LN2
                st = stats_pool.tile([128, 2, 6], f32, tag="bn")
                for g in range(2):
                    nc.vector.bn_stats(out=st[:, g, :], in_=s2_tile[:, g * 512:(g + 1) * 512])
                x2_tile = cb[:, 3 * (t % 4) + 2, :]
                mv = stats_pool.tile([128, 2], f32, tag="mv")
                nc.vector.bn_aggr(out=mv[:], in_=st[:])
                rstd = stats_pool.tile([128, 1], f32, tag="rstd")
                nc.scalar.activation(out=rstd[:], in_=mv[:, 1:2], func=AF.Sqrt,
                                     bias=eps_t[:], scale=1.0)
                nc.vector.reciprocal(out=rstd[:], in_=rstd[:])
                nc.vector.tensor_scalar(out=x2_tile, in0=s2_tile,
                                        scalar1=mv[:, 0:1], scalar2=rstd[:],
                                        op0=ALU.subtract, op1=ALU.mult)
                nc.sync.dma_start(out=x2_out[t * 128:(t + 1) * 128, :], in_=x2_tile)

    nc.compile()
    return nc


def _build_launch2():
    bass, tile, bacc, mybir, make_identity, f32, f32r, f16, AF, ALU = _builders()
    from contextlib import ExitStack

    nc = bacc.Bacc(None, target_bir_lowering=False, debug=True)

    T = nc.dram_tensor("T", [CAP, D], f32, kind="ExternalInput")
    gate = nc.dram_tensor("gate", [128, CAP // 128], f32, kind="ExternalInput")
    W1 = nc.dram_tensor("W1", [D, F], f32r, kind="ExternalInput")
    b1 = nc.dram_tensor("b1", [128, F // 128], f32, kind="ExternalInput")
    W2 = nc.dram_tensor("W2", [F, D], f16, kind="ExternalInput")
    b2row = nc.dram_tensor("b2row", [1, D], f16, kind="ExternalInput")
    y_out = nc.dram_tensor("y", [CAP, D], f32, kind="ExternalOutput")

    NFT = F // 128           # 32 F chunks
    NTT = CAP // 128         # 13 token tiles
    w1_r = W1.rearrange("(dc p) n -> p dc n", p=128)
    w2_r = W2.rearrange("(fc p) n -> p fc n", p=128)
    TCS = [(0, 640), (640, 640), (1280, 384)]
    NSPLITS = {640: [(0, 320), (320, 320)], 384: [(0, 384)]}

    with tile.TileContext(nc) as tc:
        with ExitStack() as ctx:
            ctx.enter_context(nc.allow_low_precision(reason="fp32r/fp16 matmuls"))
            persist = ctx.enter_context(tc.tile_pool(name="persist", bufs=1))
            scratch = ctx.enter_context(tc.tile_pool(name="scratch", bufs=2))
            small = ctx.enter_context(tc.tile_pool(name="small", bufs=1))
            wpool = ctx.enter_context(tc.tile_pool(name="wpool", bufs=3))
            ypool = ctx.enter_context(tc.tile_pool(name="ypool", bufs=2))
            ps_pp = ctx.enter_context(tc.tile_pool(name="ps_pp", bufs=3, space="PSUM"))
            ps_y = ctx.enter_context(tc.tile_pool(name="ps_y", bufs=3, space="PSUM"))

            ident32 = small.tile([128, 128], f32)
            make_identity(nc, ident32)
            ident = small.tile([128, 128], f32r)
            nc.vector.tensor_copy(ident[:], ident32[:])
            onesf = small.tile([1, 128], f32)
            nc.vector.memset(onesf[:], 1.0)
            ones16 = small.tile([1, 128], f16)
            nc.vector.tensor_copy(ones16[:], onesf[:])
            gate_sb = small.tile([128, NTT], f32)
            nc.sync.dma_start(out=gate_sb[:], in_=gate[:])
            b1_sb = small.tile([128, NFT], f32)
            nc.sync.dma_start(out=b1_sb[:], in_=b1[:])
            b2_sb = small.tile([1, D], f16)
            nc.sync.dma_start(out=b2_sb[:], in_=b2row[:])

            # transpose all tokens -> TT [128, 8, CAP] (feature-major)
            TT = persist.tile([128, 8, CAP], f32r)
            for t in range(NTT):
                s_tile = scratch.tile([128, D], f32, tag="ld")
                nc.sync.dma_start(out=s_tile[:], in_=T[t * 128:(t + 1) * 128, :])
                xr_tile = scratch.tile([128, D], f32r, tag="xr")
                nc.vector.tensor_copy(xr_tile[:], s_tile[:])
                for dg in range(2):
                    pt = ps_pp.tile([128, 512], f32r, tag="pp", name="pt")
                    for d4 in range(4):
                        d = dg * 4 + d4
                        nc.tensor.transpose(pt[:, d4 * 128:(d4 + 1) * 128],
                                            xr_tile[:, d * 128:(d + 1) * 128], ident[:])
                    nc.vector.tensor_copy(
                        out=TT[:, dg * 4:(dg + 1) * 4, t * 128:(t + 1) * 128],
                        in_=pt.rearrange("p (a b) -> p a b", a=4))

            # W2 cached fully in fp16 [128, 32, 1024]
            w2_sb = persist.tile([128, NFT, D], f16)
            nc.sync.dma_start(out=w2_sb[:], in_=w2_r[:])

            hT = persist.tile([128, NFT, 640], f16)
            for tc0, tcn in TCS:
                for fc in range(NFT):
                    w1_t = wpool.tile([128, 8, 128], f32r, tag="w1")
                    nc.sync.dma_start(out=w1_t[:], in_=w1_r[:, :, fc * 128:(fc + 1) * 128])
                    for n0, nn in NSPLITS[tcn]:
                        ps = ps_pp.tile([128, 512], f32, tag="pp", name="ps_h")
                        for dc in range(8):
                            nc.tensor.matmul(ps[:, :nn], w1_t[:, dc, :],
                                             TT[:, dc, tc0 + n0:tc0 + n0 + nn],
                                             start=(dc == 0), stop=(dc == 7))
                        nc.scalar.activation(out=hT[:, fc, n0:n0 + nn], in_=ps[:, :nn],
                                             func=AF.Relu, bias=b1_sb[:, fc:fc + 1],
                                             scale=1.0)
                nsub = tcn // 128
                for sub in range(nsub):
                    g_idx = tc0 // 128 + sub
                    y_tile = ypool.tile([128, D], f32, tag="y")
                    for ncol in range(2):
                        py = ps_y.tile([128, 512], f32, tag="py")
                        for fc in range(NFT):
                            nc.tensor.matmul(py[:], hT[:, fc, sub * 128:(sub + 1) * 128],
                                             w2_sb[:, fc, ncol * 512:(ncol + 1) * 512],
                                             start=(fc == 0), stop=False)
                        nc.tensor.matmul(py[:], ones16[:, 0:128],
                                         b2_sb[:, ncol * 512:(ncol + 1) * 512],
                                         start=False, stop=True)
                        nc.vector.tensor_scalar_mul(
                            out=y_tile[:, ncol * 512:(ncol + 1) * 512], in0=py[:],
                            scalar1=gate_sb[:, g_idx:g_idx + 1])
                    nc.sync.dma_start(out=y_out[g_idx * 128:(g_idx + 1) * 128, :],
                                      in_=y_tile[:])

    nc.compile()
    return nc


def _get_ncs():
    if "nc1" not in _CACHE:
        _CACHE["nc1"] = _build_launch1()
    if "nc2" not in _CACHE:
        _CACHE["nc2"] = _build_launch2()
    return _CACHE["nc1"], _CACHE["nc2"]


def _numpy_fallback(src, pad_mask, g1, be1, Wq, bq, Wk, bk, Wv, bv, Wo, bo,
                    g2, be2, Wr, br, W1e, b1e, W2e, b2e):
    """Pure numpy reference (only used if inputs deviate from expected layout)."""
    def ln(x, g, b):
        mu = x.mean(-1, keepdims=True)
        var = ((x - mu) ** 2).mean(-1, keepdims=True)
        return (x - mu) / np.sqrt(var + EPS) * g + b

    x = ln(src, g1, be1)
    q = (x @ Wq + bq).reshape(B, S, H, DH)
    k = (x @ Wk + bk).reshape(B, S, H, DH)
    v = (x @ Wv + bv).reshape(B, S, H, DH)
    scores = np.einsum("bqhd,bkhd->bhqk", q, k) / np.sqrt(np.float32(DH))
    scores = np.where(pad_mask[:, None, None, :], -np.inf, scores)
    scores -= scores.max(-1, keepdims=True)
    attn = np.exp(scores)
    attn /= attn.sum(-1, keepdims=True)
    o = np.einsum("bhqk,bkhd->bqhd", attn, v).reshape(B, S, D)
    src = src + o @ Wo + bo
    token_mask = ~pad_mask
    x = ln(src, g2, be2)
    logits = x @ Wr + br
    p = np.exp(logits - logits.max(-1, keepdims=True))
    p /= p.sum(-1, keepdims=True)
    gate = p.max(-1)
    idx = p.argmax(-1)
    moe = np.zeros_like(x)
    for e in range(E):
        m = (idx == e) & token_mask
        h = np.maximum(x @ W1e[e] + b1e[e], 0.0)
        y = h @ W2e[e] + b2e[e]
        moe = moe + np.where(m[..., None], gate[..., None] * y, 0.0)
    return (src + moe).astype(np.float32)


def kernel(src, pad_mask, g1, be1, Wq, bq, Wk, bk, Wv, bv, Wo, bo,
           g2, be2, Wr, br, W1e, b1e, W2e, b2e):
    from concourse.bass_utils import run_bass_kernel_spmd

    src = np.asarray(src, dtype=np.float32)
    pad_mask = np.asarray(pad_mask)
    args32 = [np.asarray(a, dtype=np.float32) for a in
              (g1, be1, Wq, bq, Wk, bk, Wv, bv, Wo, bo, g2, be2, Wr, br,
               W1e, b1e, W2e, b2e)]
    (g1, be1, Wq, bq, Wk, bk, Wv, bv, Wo, bo, g2, be2, Wr, br,
     W1e, b1e, W2e, b2e) = args32

    expected_mask = np.broadcast_to(np.arange(S)[None, :] >= SR, (B, S))
    if src.shape != (B, S, D) or not np.array_equal(pad_mask, expected_mask):
        return _numpy_fallback(src, pad_mask, g1, be1, Wq, bq, Wk, bk, Wv, bv,
                               Wo, bo, g2, be2, Wr, br, W1e, b1e, W2e, b2e)

    nc1, nc2 = _get_ncs()

    # fold LN scale/bias into the following projections
    Wq_f = g1[:, None] * Wq
    Wk_f = g1[:, None] * Wk
    Wv_f = g1[:, None] * Wv
    bq_f = bq + be1 @ Wq
    bk_f = bk + be1 @ Wk
    bv_f = bv + be1 @ Wv
    Wr_f = g2[:, None] * Wr
    br_f = br + be2 @ Wr
    W1_f = W1e * g2[None, :, None]                       # [E, D, F]
    b1_f = b1e + np.einsum("d,edf->ef", be2, W1e)        # [E, F]

    bqk_np = np.stack([bq_f.reshape(8, 128).T, bk_f.reshape(8, 128).T], axis=2)
    bqk_np = np.ascontiguousarray(bqk_np, dtype=np.float32)  # [128, 8, 2]

    in_maps1 = []
    for b in range(B):
        in_maps1.append({
            "src": src[b],
            "srcbo": src[b] + bo[None, :],
            "Wq": Wq_f, "Wk": Wk_f, "Wv": Wv_f, "Wo": Wo,
            "bqk": bqk_np,
            "bv_row": np.ascontiguousarray(bv_f[None, :]),
        })
    res1 = run_bass_kernel_spmd(nc1, in_maps1, list(range(NCORE))).results
    src2 = np.stack([res1[b]["src2"] for b in range(B)])    # [B, S, D]
    x2 = np.stack([res1[b]["x2"] for b in range(B)])        # [B, S, D]

    # ---- host routing (all-to-all dispatch) ----
    x2_flat = x2.reshape(B * S, D)
    logits = x2_flat @ Wr_f + br_f
    lmax = logits.max(-1, keepdims=True)
    p = np.exp(logits - lmax)
    p /= p.sum(-1, keepdims=True)
    gate_all = p.max(-1)
    idx_all = p.argmax(-1)
    real = (~expected_mask).reshape(-1)

    ids_per_e = []
    for e in range(E):
        ids = np.nonzero((idx_all == e) & real)[0]
        ids_per_e.append(ids)

    in_maps2 = []
    for e in range(E):
        ids = ids_per_e[e][:CAP]
        Te = np.zeros((CAP, D), dtype=np.float32)
        Te[:len(ids)] = x2_flat[ids]
        ge = np.zeros(CAP, dtype=np.float32)
        ge[:len(ids)] = gate_all[ids]
        in_maps2.append({
            "T": Te,
            "gate": np.ascontiguousarray(ge.reshape(CAP // 128, 128).T),
            "W1": np.ascontiguousarray(W1_f[e]),
            "b1": np.ascontiguousarray(b1_f[e].reshape(F // 128, 128).T),
            "W2": np.ascontiguousarray(W2e[e].astype(np.float16)),
            "b2row": np.ascontiguousarray(b2e[e][None, :].astype(np.float16)),
        })
    res2 = run_bass_kernel_spmd(nc2, in_maps2, list(range(NCORE))).results

    out = src2.reshape(B * S, D).copy()
    for e in range(E):
        ids = ids_per_e[e]
        n = min(len(ids), CAP)
        out[ids[:n]] += res2[e]["y"][:n]
        if len(ids) > CAP:  # capacity overflow: host fallback for the tail
            ids_t = ids[CAP:]
            h = np.maximum(x2_flat[ids_t] @ W1_f[e] + b1_f[e], 0.0)
            y = h @ W2e[e] + b2e[e]
            out[ids_t] += gate_all[ids_t, None] * y

    return out.reshape(B, S, D).astype(np.float32)


# revision 22
# speedup vs baseline: 11769.8249x; 11769.8249x over previous
"""Trainium2 Bass kernel for nn_EncoderLayer (attention + top-1 Switch MoE).

Strategy (8 NeuronCores):
  Launch 1 (data-parallel over batch): per core one batch row — LN1, QKV in
    transposed (feature-major) layout, flash-style attention with padded keys
    skipped, out-proj + residual, LN2. fp32r matmuls (full PE rate, ~1e-4).
  Host: router softmax/argmax (the "all-to-all" token dispatch).
  Launch 2 (expert-parallel): core e runs expert e's FFN on its routed
    tokens (capacity 1664 = 13*128), gate-scaled. Host scatter-adds back.
"""

import os
import numpy as np

os.environ.setdefault("MYCRO_LOCAL_CACHE", "1")

B, S, D, H, E, F = 8, 2048, 1024, 16, 8, 4096
DH = D // H          # 64
SR = (3 * S) // 4    # 1536 real tokens per sequence
NCORE = 8
CAP = 1664           # expert capacity (13*128)
EPS = 1e-5

_CACHE = {}


def _builders():
    import concourse.bass as bass
    import concourse.tile as tile
    from concourse import bacc, mybir
    from concourse.masks import make_identity

    f32 = mybir.dt.float32
    f32r = mybir.dt.float32r
    f16 = mybir.dt.float16
    AF = mybir.ActivationFunctionType
    ALU = mybir.AluOpType
    return bass, tile, bacc, mybir, make_identity, f32, f32r, f16, AF, ALU


def _build_launch1():
    bass, tile, bacc, mybir, make_identity, f32, f32r, f16, AF, ALU = _builders()
    from contextlib import ExitStack

    nc = bacc.Bacc(None, target_bir_lowering=False, debug=True)

    src = nc.dram_tensor("src", [S, D], f32, kind="ExternalInput")
    srcbo = nc.dram_tensor("srcbo", [S, D], f32, kind="ExternalInput")  # src + bo
    Wq = nc.dram_tensor("Wq", [8, 128, 8, 128], f32r, kind="ExternalInput")
    Wk = nc.dram_tensor("Wk", [8, 128, 8, 128], f32r, kind="ExternalInput")
    Wv = nc.dram_tensor("Wv", [D, D], f32r, kind="ExternalInput")
    Wo = nc.dram_tensor("Wo", [D, D], f32r, kind="ExternalInput")
    bqk = nc.dram_tensor("bqk", [128, 8, 2], f32, kind="ExternalInput")
    bv_row = nc.dram_tensor("bv_row", [1, D], f32r, kind="ExternalInput")
    src2_out = nc.dram_tensor("src2", [S, D], f32, kind="ExternalOutput")
    x2_out = nc.dram_tensor("x2", [S, D], f32, kind="ExternalOutput")
    oT_spill = nc.dram_tensor("oT_spill", [S // 128, 128, 8, 128], f32r)
    rec_dram = nc.dram_tensor("rec_dram", [8, 4, 2, 512], f32r)

    NT = S // 128            # 16 token tiles
    NKT = SR // 128          # 12 key chunks
    NQC = S // 512           # 4 q chunks

    wv_r = Wv.rearrange("(dc p) n -> p dc n", p=128)
    wo_r = Wo.rearrange("(dc p) n -> p dc n", p=128)

    with tile.TileContext(nc) as tc:
        with ExitStack() as ctx:
            ctx.enter_context(nc.allow_low_precision(reason="fp32r matmul pipeline"))
            persist = ctx.enter_context(tc.tile_pool(name="persist", bufs=1))
            scratch = ctx.enter_context(tc.tile_pool(name="scratch", bufs=2))
            small = ctx.enter_context(tc.tile_pool(name="small", bufs=1))
            stats_pool = ctx.enter_context(tc.tile_pool(name="stats", bufs=4))
            qpool = ctx.enter_context(tc.tile_pool(name="qpool", bufs=2))
            kpool = ctx.enter_context(tc.tile_pool(name="kpool", bufs=1))
            wvpool = ctx.enter_context(tc.tile_pool(name="wvpool", bufs=1))
            wpool = ctx.enter_context(tc.tile_pool(name="wqk", bufs=2))
            ppool = ctx.enter_context(tc.tile_pool(name="ppool", bufs=2))
            npool = ctx.enter_context(tc.tile_pool(name="npool", bufs=2))
            vpool = ctx.enter_context(tc.tile_pool(name="vpool", bufs=1))
            opool = ctx.enter_context(tc.tile_pool(name="opool", bufs=2))
            olpool = ctx.enter_context(tc.tile_pool(name="olpool", bufs=2))
            ps_pp = ctx.enter_context(tc.tile_pool(name="ps_pp", bufs=2, space="PSUM"))
            ps_s = ctx.enter_context(tc.tile_pool(name="ps_s", bufs=2, space="PSUM"))
            ps_ob = ctx.enter_context(tc.tile_pool(name="ps_ob", bufs=2, space="PSUM"))

            # ---- constants ----
            ident32 = small.tile([128, 128], f32)
            make_identity(nc, ident32)
            ident = small.tile([128, 128], f32r)
            nc.vector.tensor_copy(ident[:], ident32[:])
            ones32 = small.tile([1, 128], f32)
            nc.vector.memset(ones32[:], 1.0)
            ones_tok = small.tile([1, 128], f32r)
            nc.vector.tensor_copy(ones_tok[:], ones32[:])
            ones96 = small.tile([128, 96], f32)
            nc.vector.memset(ones96[:], 1.0)
            eps_t = small.tile([128, 1], f32)
            nc.vector.memset(eps_t[:], EPS)
            bqk_sb = small.tile([128, 8, 2], f32)
            nc.sync.dma_start(out=bqk_sb[:], in_=bqk[:])
            bv_sb = small.tile([1, D], f32r)
            nc.sync.dma_start(out=bv_sb[:], in_=bv_row[:])

            # ---- phase A: LN1 + transpose -> xT [128, 8, S] ----
            xT = persist.tile([128, 8, S], f32r)
            for t in range(NT):
                s_tile = scratch.tile([128, D], f32, tag="ld")
                nc.sync.dma_start(out=s_tile[:], in_=src[t * 128:(t + 1) * 128, :])
                st = stats_pool.tile([128, 2, 6], f32, tag="bn")
                for g in range(2):
                    nc.vector.bn_stats(out=st[:, g, :], in_=s_tile[:, g * 512:(g + 1) * 512])
                mv = stats_pool.tile([128, 2], f32, tag="mv")
                nc.vector.bn_aggr(out=mv[:], in_=st[:])
                rstd = stats_pool.tile([128, 1], f32, tag="rstd")
                nc.scalar.activation(out=rstd[:], in_=mv[:, 1:2], func=AF.Sqrt,
                                     bias=eps_t[:], scale=1.0)
                nc.vector.reciprocal(out=rstd[:], in_=rstd[:])
                x_tile = scratch.tile([128, D], f32r, tag="x")
                nc.vector.tensor_scalar(out=x_tile[:], in0=s_tile[:],
                                        scalar1=mv[:, 0:1], scalar2=rstd[:],
                                        op0=ALU.subtract, op1=ALU.mult)
                for dg in range(2):
                    pt = ps_pp.tile([128, 512], f32r, tag="pp", name="pt")
                    for d4 in range(4):
                        d = dg * 4 + d4
                        nc.tensor.transpose(pt[:, d4 * 128:(d4 + 1) * 128],
                                            x_tile[:, d * 128:(d + 1) * 128], ident[:])
                    ev_eng = nc.vector.tensor_copy if dg == 0 else nc.scalar.copy
                    ev_eng(
                        out=xT[:, dg * 4:(dg + 1) * 4, t * 128:(t + 1) * 128],
                        in_=pt.rearrange("p (a b) -> p a b", a=4))

            # ---- phase B: attention per quad / head pair ----
            for quad in range(2):
                # V + bias for 8 heads of this quad -> Vaug [128, 12, 8, 65]
                wv_sb = wvpool.tile([128, 8, 512], f32r, tag="wv")
                nc.sync.dma_start(out=wv_sb[:], in_=wv_r[:, :, quad * 512:(quad + 1) * 512])
                vaug = vpool.tile([128, NKT, 8, 65], f32r, tag="vaug")
                for vt in range(NKT):
                    ps = ps_pp.tile([128, 512], f32, tag="pp", name="ps_v")
                    for dc in range(8):
                        nc.tensor.matmul(ps[:], xT[:, dc, vt * 128:(vt + 1) * 128],
                                         wv_sb[:, dc, :], start=(dc == 0), stop=False)
                    nc.tensor.matmul(ps[:], ones_tok[:, 0:128],
                                     bv_sb[:, quad * 512:(quad + 1) * 512],
                                     start=False, stop=True)
                    nc.vector.tensor_copy(out=vaug[:, vt, :, 0:64],
                                          in_=ps.rearrange("p (h d) -> p h d", h=8))
                nc.vector.tensor_copy(
                    out=vaug[:, :, :, 64:65].rearrange("p a b one -> p (a b one)"),
                    in_=ones96[:])

                for cc in range(4):
                    c = quad * 4 + cc
                    wq_t = wpool.tile([128, 8, 128], f32r, tag="wq")
                    nc.sync.dma_start(out=wq_t[:], in_=Wq[c])
                    wk_t = wpool.tile([128, 8, 128], f32r, tag="wk")
                    nc.sync.dma_start(out=wk_t[:], in_=Wk[c])

                    qt = qpool.tile([128, S], f32r, tag="qt")
                    for qc in range(NQC):
                        ps = ps_pp.tile([128, 512], f32, tag="pp", name="ps_q")
                        for dc in range(8):
                            nc.tensor.matmul(ps[:], wq_t[:, dc, :],
                                             xT[:, dc, qc * 512:(qc + 1) * 512],
                                             start=(dc == 0), stop=(dc == 7))
                        nc.vector.tensor_scalar_add(out=qt[:, qc * 512:(qc + 1) * 512],
                                                    in0=ps[:], scalar1=bqk_sb[:, c, 0:1])
                    kt = kpool.tile([128, SR], f32r, tag="kt")
                    for kc3 in range(SR // 512):
                        ps = ps_pp.tile([128, 512], f32, tag="pp", name="ps_k")
                        for dc in range(8):
                            nc.tensor.matmul(ps[:], wk_t[:, dc, :],
                                             xT[:, dc, kc3 * 512:(kc3 + 1) * 512],
                                             start=(dc == 0), stop=(dc == 7))
                        nc.vector.tensor_scalar_add(out=kt[:, kc3 * 512:(kc3 + 1) * 512],
                                                    in0=ps[:], scalar1=bqk_sb[:, c, 1:2])

                    for qc in range(NQC):
                        po = [ps_ob.tile([128, 512], f32, tag="po", name=f"po{_h}")
                              for _h in range(2)]
                        for kc in range(NKT):
                            sp = ps_s.tile([128, 2, 512], f32, tag="ss", name="sp")
                            for half in range(2):
                                nc.tensor.matmul(
                                    sp[:, half, :],
                                    kt[64 * half:64 * half + 64, kc * 128:(kc + 1) * 128],
                                    qt[64 * half:64 * half + 64, qc * 512:(qc + 1) * 512],
                                    start=True, stop=True)
                            pt = ppool.tile([128, 2, 512], f32r, tag="pT")
                            nc.scalar.activation(
                                out=pt.rearrange("p a b -> p (a b)"),
                                in_=sp.rearrange("p a b -> p (a b)"),
                                func=AF.Exp, scale=0.125)
                            for half in range(2):
                                nc.tensor.matmul(
                                    po[half][0:65, :],
                                    vaug[:, kc, 2 * cc + half, 0:65],
                                    pt[:, half, :],
                                    start=(kc == 0), stop=(kc == NKT - 1))
                        for half in range(2):
                            raw = opool.tile([65, 512], f32r, tag="raw")
                            nc.vector.tensor_copy(out=raw[:], in_=po[half][0:65, :])
                            rs8 = npool.tile([8, 64], f32r, tag="rs8")
                            nc.sync.dma_start(out=rs8[:], in_=raw[64:65, :])
                            rr8 = npool.tile([8, 64], f32r, tag="rr8")
                            nc.vector.reciprocal(out=rr8[:], in_=rs8[:])
                            nc.sync.dma_start(out=rec_dram[c, qc, half], in_=rr8[:])
                            sbc = npool.tile([64, 512], f32r, tag="sbc")
                            nc.sync.dma_start(
                                out=sbc[:],
                                in_=rec_dram[c, qc, half].partition_broadcast(64))
                            nc.vector.tensor_mul(raw[0:64, :], raw[0:64, :], sbc[:])
                            nc.sync.dma_start(
                                out=oT_spill[4 * qc:4 * qc + 4,
                                             64 * half:64 * half + 64,
                                             c, :].rearrange("j p t -> p j t"),
                                in_=raw[0:64, :])

            # ---- phase C: out-projection + residual + LN2 ----
            wo_sb4 = vpool.tile([128, 8, D], f32r, tag="vaug", name="wo_sb4")
            nc.sync.dma_start(out=wo_sb4[:], in_=wo_r[:])
            cbuf = persist.tile([128, 8, S], f32, tag="xT", name="cbuf")
            cb = cbuf.rearrange("p a b -> p (a b)").rearrange("p (j d) -> p j d", d=D)
            for t in range(NT):
                res_tile = cb[:, 3 * (t % 4) + 0, :]
                nc.sync.dma_start(out=res_tile, in_=srcbo[t * 128:(t + 1) * 128, :])
                pss = ps_s.tile([128, 2, 512], f32, tag="ss", name="ps_op")
                ot_l = olpool.tile([128, 8, 128], f32r, tag="otl")
                nc.sync.dma_start(out=ot_l[:], in_=oT_spill[t])
                for dc in range(8):
                    for ncol in range(2):
                        nc.tensor.matmul(pss[:, ncol, :], ot_l[:, dc, :],
                                         wo_sb4[:, dc, ncol * 512:(ncol + 1) * 512],
                                         start=(dc == 0), stop=(dc == 7))
                s2_tile = cb[:, 3 * (t % 4) + 1, :]
                nc.vector.tensor_add(s2_tile,
                                     pss.rearrange("p a b -> p (a b)"),
                                     res_tile)
                nc.sync.dma_start(out=src2_out[t * 128:(t + 1) * 128, :], in_=s2_tile)
                # LN2
                st = stats_pool.tile([128, 2, 6], f32, tag="bn")
                for g in range(2):
                    nc.vector.bn_stats(out=st[:, g, :], in_=s2_tile[:, g * 512:(g + 1) * 512])
                x2_tile = cb[:, 3 * (t % 4) + 2, :]
                mv = stats_pool.tile([128, 2], f32, tag="mv")
                nc.vector.bn_aggr(out=mv[:], in_=st[:])
                rstd = stats_pool.tile([128, 1], f32, tag="rstd")
                nc.scalar.activation(out=rstd[:], in_=mv[:, 1:2], func=AF.Sqrt,
                                     bias=eps_t[:], scale=1.0)
                nc.vector.reciprocal(out=rstd[:], in_=rstd[:])
                nc.vector.tensor_scalar(out=x2_tile, in0=s2_tile,
                                        scalar1=mv[:, 0:1], scalar2=rstd[:],
                                        op0=ALU.subtract, op1=ALU.mult)
                nc.sync.dma_start(out=x2_out[t * 128:(t + 1) * 128, :], in_=x2_tile)

    nc.compile()
    return nc


def _build_launch2():
    bass, tile, bacc, mybir, make_identity, f32, f32r, f16, AF, ALU = _builders()
    from contextlib import ExitStack

    nc = bacc.Bacc(None, target_bir_lowering=False, debug=True)

    T = nc.dram_tensor("T", [CAP, D], f32, kind="ExternalInput")
    gate = nc.dram_tensor("gate", [128, CAP // 128], f32, kind="ExternalInput")
    W1 = nc.dram_tensor("W1", [F // 128, 128, 8, 128], f32r, kind="ExternalInput")
    b1 = nc.dram_tensor("b1", [128, F // 128], f32, kind="ExternalInput")
    W2 = nc.dram_tensor("W2", [F, D], f16, kind="ExternalInput")
    b2row = nc.dram_tensor("b2row", [1, D], f16, kind="ExternalInput")
    y_out = nc.dram_tensor("y", [CAP, D], f32, kind="ExternalOutput")

    NFT = F // 128           # 32 F chunks
    NTT = CAP // 128         # 13 token tiles
    w2_r = W2.rearrange("(fc p) n -> p fc n", p=128)
    TCS = [(0, 640), (640, 640), (1280, 384)]
    NSPLITS = {640: [(0, 320), (320, 320)], 384: [(0, 384)]}

    with tile.TileContext(nc) as tc:
        with ExitStack() as ctx:
            ctx.enter_context(nc.allow_low_precision(reason="fp32r/fp16 matmuls"))
            persist = ctx.enter_context(tc.tile_pool(name="persist", bufs=1))
            scratch = ctx.enter_context(tc.tile_pool(name="scratch", bufs=2))
            small = ctx.enter_context(tc.tile_pool(name="small", bufs=1))
            wpool = ctx.enter_context(tc.tile_pool(name="wpool", bufs=3))
            ypool = ctx.enter_context(tc.tile_pool(name="ypool", bufs=2))
            ps_pp = ctx.enter_context(tc.tile_pool(name="ps_pp", bufs=3, space="PSUM"))
            ps_y = ctx.enter_context(tc.tile_pool(name="ps_y", bufs=3, space="PSUM"))

            ident32 = small.tile([128, 128], f32)
            make_identity(nc, ident32)
            ident = small.tile([128, 128], f32r)
            nc.vector.tensor_copy(ident[:], ident32[:])
            onesf = small.tile([1, 128], f32)
            nc.vector.memset(onesf[:], 1.0)
            ones16 = small.tile([1, 128], f16)
            nc.vector.tensor_copy(ones16[:], onesf[:])
            gate_sb = small.tile([128, NTT], f32)
            nc.sync.dma_start(out=gate_sb[:], in_=gate[:])
            b1_sb = small.tile([128, NFT], f32)
            nc.sync.dma_start(out=b1_sb[:], in_=b1[:])
            b2_sb = small.tile([1, D], f16)
            nc.sync.dma_start(out=b2_sb[:], in_=b2row[:])

            # transpose all tokens -> TT [128, 8, CAP] (feature-major)
            TT = persist.tile([128, 8, CAP], f32r)
            for t in range(NTT):
                s_tile = scratch.tile([128, D], f32, tag="ld")
                nc.sync.dma_start(out=s_tile[:], in_=T[t * 128:(t + 1) * 128, :])
                xr_tile = scratch.tile([128, D], f32r, tag="xr")
                nc.vector.tensor_copy(xr_tile[:], s_tile[:])
                for dg in range(2):
                    pt = ps_pp.tile([128, 512], f32r, tag="pp", name="pt")
                    for d4 in range(4):
                        d = dg * 4 + d4
                        nc.tensor.transpose(pt[:, d4 * 128:(d4 + 1) * 128],
                                            xr_tile[:, d * 128:(d + 1) * 128], ident[:])
                    nc.vector.tensor_copy(
                        out=TT[:, dg * 4:(dg + 1) * 4, t * 128:(t + 1) * 128],
                        in_=pt.rearrange("p (a b) -> p a b", a=4))

            # W2 cached fully in fp16 [128, 32, 1024]
            w2_sb = persist.tile([128, NFT, D], f16)
            for _wc in range(8):
                nc.sync.dma_start(out=w2_sb[:, _wc * 4:(_wc + 1) * 4, :],
                                  in_=w2_r[:, _wc * 4:(_wc + 1) * 4, :])

            hT = persist.tile([128, NFT, 640], f16)
            for tc0, tcn in TCS:
                for fc in range(NFT):
                    w1_t = wpool.tile([128, 8, 128], f32r, tag="w1")
                    nc.sync.dma_start(out=w1_t[:], in_=W1[fc])
                    for n0, nn in NSPLITS[tcn]:
                        ps = ps_pp.tile([128, 512], f32, tag="pp", name="ps_h")
                        for dc in range(8):
                            nc.tensor.matmul(ps[:, :nn], w1_t[:, dc, :],
                                             TT[:, dc, tc0 + n0:tc0 + n0 + nn],
                                             start=(dc == 0), stop=(dc == 7))
                        nc.scalar.activation(out=hT[:, fc, n0:n0 + nn], in_=ps[:, :nn],
                                             func=AF.Relu, bias=b1_sb[:, fc:fc + 1],
                                             scale=1.0)
                nsub = tcn // 128
                for sub in range(nsub):
                    g_idx = tc0 // 128 + sub
                    y_tile = ypool.tile([128, D], f32, tag="y")
                    for ncol in range(2):
                        py = ps_y.tile([128, 512], f32, tag="py")
                        for fc in range(NFT):
                            nc.tensor.matmul(py[:], hT[:, fc, sub * 128:(sub + 1) * 128],
                                             w2_sb[:, fc, ncol * 512:(ncol + 1) * 512],
                                             start=(fc == 0), stop=False)
                        nc.tensor.matmul(py[:], ones16[:, 0:128],
                                         b2_sb[:, ncol * 512:(ncol + 1) * 512],
                                         start=False, stop=True)
                        nc.vector.tensor_scalar_mul(
                            out=y_tile[:, ncol * 512:(ncol + 1) * 512], in0=py[:],
                            scalar1=gate_sb[:, g_idx:g_idx + 1])
                    nc.sync.dma_start(out=y_out[g_idx * 128:(g_idx + 1) * 128, :],
                                      in_=y_tile[:])

    nc.compile()
    return nc


def _get_ncs():
    if "nc1" not in _CACHE:
        _CACHE["nc1"] = _build_launch1()
    if "nc2" not in _CACHE:
        _CACHE["nc2"] = _build_launch2()
    return _CACHE["nc1"], _CACHE["nc2"]


def _numpy_fallback(src, pad_mask, g1, be1, Wq, bq, Wk, bk, Wv, bv, Wo, bo,
                    g2, be2, Wr, br, W1e, b1e, W2e, b2e):
    """Pure numpy reference (only used if inputs deviate from expected layout)."""
    def ln(x, g, b):
        mu = x.mean(-1, keepdims=True)
        var = ((x - mu) ** 2).mean(-1, keepdims=True)
        return (x - mu) / np.sqrt(var + EPS) * g + b

    x = ln(src, g1, be1)
    q = (x @ Wq + bq).reshape(B, S, H, DH)
    k = (x @ Wk + bk).reshape(B, S, H, DH)
    v = (x @ Wv + bv).reshape(B, S, H, DH)
    scores = np.einsum("bqhd,bkhd->bhqk", q, k) / np.sqrt(np.float32(DH))
    scores = np.where(pad_mask[:, None, None, :], -np.inf, scores)
    scores -= scores.max(-1, keepdims=True)
    attn = np.exp(scores)
    attn /= attn.sum(-1, keepdims=True)
    o = np.einsum("bhqk,bkhd->bqhd", attn, v).reshape(B, S, D)
    src = src + o @ Wo + bo
    token_mask = ~pad_mask
    x = ln(src, g2, be2)
    logits = x @ Wr + br
    p = np.exp(logits - logits.max(-1, keepdims=True))
    p /= p.sum(-1, keepdims=True)
    gate = p.max(-1)
    idx = p.argmax(-1)
    moe = np.zeros_like(x)
    for e in range(E):
        m = (idx == e) & token_mask
        h = np.maximum(x @ W1e[e] + b1e[e], 0.0)
        y = h @ W2e[e] + b2e[e]
        moe = moe + np.where(m[..., None], gate[..., None] * y, 0.0)
    return (src + moe).astype(np.float32)


def _tile_dxd(w):
    # [D, D] -> [8(c), 128(p), 8(dc), 128(n)]: w[dc*128+p, c*128+n]
    return np.ascontiguousarray(
        w.reshape(8, 128, 8, 128).transpose(2, 1, 0, 3))


def _tile_w1(w):
    # [D, F] -> [32(fc), 128(p), 8(dc), 128(n)]: w[dc*128+p, fc*128+n]
    return np.ascontiguousarray(
        w.reshape(8, 128, 32, 128).transpose(2, 1, 0, 3))


def kernel(src, pad_mask, g1, be1, Wq, bq, Wk, bk, Wv, bv, Wo, bo,
           g2, be2, Wr, br, W1e, b1e, W2e, b2e):
    from concourse.bass_utils import run_bass_kernel_spmd

    src = np.asarray(src, dtype=np.float32)
    pad_mask = np.asarray(pad_mask)
    args32 = [np.asarray(a, dtype=np.float32) for a in
              (g1, be1, Wq, bq, Wk, bk, Wv, bv, Wo, bo, g2, be2, Wr, br,
               W1e, b1e, W2e, b2e)]
    (g1, be1, Wq, bq, Wk, bk, Wv, bv, Wo, bo, g2, be2, Wr, br,
     W1e, b1e, W2e, b2e) = args32

    expected_mask = np.broadcast_to(np.arange(S)[None, :] >= SR, (B, S))
    if src.shape != (B, S, D) or not np.array_equal(pad_mask, expected_mask):
        return _numpy_fallback(src, pad_mask, g1, be1, Wq, bq, Wk, bk, Wv, bv,
                               Wo, bo, g2, be2, Wr, br, W1e, b1e, W2e, b2e)

    nc1, nc2 = _get_ncs()

    # fold LN scale/bias into the following projections
    Wq_f = g1[:, None] * Wq
    Wk_f = g1[:, None] * Wk
    Wv_f = g1[:, None] * Wv
    bq_f = bq + be1 @ Wq
    bk_f = bk + be1 @ Wk
    bv_f = bv + be1 @ Wv
    Wr_f = g2[:, None] * Wr
    br_f = br + be2 @ Wr
    W1_f = W1e * g2[None, :, None]                       # [E, D, F]
    b1_f = b1e + np.einsum("d,edf->ef", be2, W1e)        # [E, F]

    bqk_np = np.stack([bq_f.reshape(8, 128).T, bk_f.reshape(8, 128).T], axis=2)
    bqk_np = np.ascontiguousarray(bqk_np, dtype=np.float32)  # [128, 8, 2]

    in_maps1 = []
    for b in range(B):
        in_maps1.append({
            "src": src[b],
            "srcbo": src[b] + bo[None, :],
            "Wq": _tile_dxd(Wq_f), "Wk": _tile_dxd(Wk_f), "Wv": Wv_f, "Wo": Wo,
            "bqk": bqk_np,
            "bv_row": np.ascontiguousarray(bv_f[None, :]),
        })
    res1 = run_bass_kernel_spmd(nc1, in_maps1, list(range(NCORE))).results
    src2 = np.stack([res1[b]["src2"] for b in range(B)])    # [B, S, D]
    x2 = np.stack([res1[b]["x2"] for b in range(B)])        # [B, S, D]

    # ---- host routing (all-to-all dispatch) ----
    x2_flat = x2.reshape(B * S, D)
    logits = x2_flat @ Wr_f + br_f
    lmax = logits.max(-1, keepdims=True)
    p = np.exp(logits - lmax)
    p /= p.sum(-1, keepdims=True)
    gate_all = p.max(-1)
    idx_all = p.argmax(-1)
    real = (~expected_mask).reshape(-1)

    ids_per_e = []
    for e in range(E):
        ids = np.nonzero((idx_all == e) & real)[0]
        ids_per_e.append(ids)

    in_maps2 = []
    for e in range(E):
        ids = ids_per_e[e][:CAP]
        Te = np.zeros((CAP, D), dtype=np.float32)
        Te[:len(ids)] = x2_flat[ids]
        ge = np.zeros(CAP, dtype=np.float32)
        ge[:len(ids)] = gate_all[ids]
        in_maps2.append({
            "T": Te,
            "gate": np.ascontiguousarray(ge.reshape(CAP // 128, 128).T),
            "W1": _tile_w1(W1_f[e]),
            "b1": np.ascontiguousarray(b1_f[e].reshape(F // 128, 128).T),
            "W2": np.ascontiguousarray(W2e[e].astype(np.float16)),
            "b2row": np.ascontiguousarray(b2e[e][None, :].astype(np.float16)),
        })
    res2 = run_bass_kernel_spmd(nc2, in_maps2, list(range(NCORE))).results

    out = src2.reshape(B * S, D).copy()
    for e in range(E):
        ids = ids_per_e[e]
        n = min(len(ids), CAP)
        out[ids[:n]] += res2[e]["y"][:n]
        if len(ids) > CAP:  # capacity overflow: host fallback for the tail
            ids_t = ids[CAP:]
            h = np.maximum(x2_flat[ids_t] @ W1_f[e] + b1_f[e], 0.0)
            y = h @ W2e[e] + b2e[e]
            out[ids_t] += gate_all[ids_t, None] * y

    return out.reshape(B, S, D).astype(np.float32)
